# revision 28
# baseline (speedup 1.0000x reference)
"""Trainium2 Bass kernel for quantized dense layer with Hadamard rotations.

Math (reference): y = (H2 @ (sq(H2@x) @ sq(w@H1)) @ H1)/4096 + bias,
sq() = per-tensor symmetric int8 stochastic quantization.

Sharding (8 cores), per the data-parallel + per-shard-Hadamard hint:
Sylvester Hadamards factor as Kronecker products; the cross-shard H32
factors are folded into the host-side shard/unshard combines, while
each core applies the per-shard H128 factors on device.  Forward: fp16
operands, H128 PE matmuls with fp32 accumulation; global quant scales
via two 1-scalar AllReduces; stochastic rounding via the fp32->int32
round-to-nearest cast with host-precomputed 0.5-noise.  The two
inverse per-shard H128 factors are folded into the quantized operands
before the GEMM (H_B (xq wq) H_F = (H_B xq)(wq H_F)) as fused
fold+transpose matmuls, so the fp16 GEMM output is final up to the
alpha scale.  Quantized activations are exchanged with a 4-way
batch-quarter AllGather pipelined against the GEMM groups; w arrives
feature-sharded so no AllToAll is needed.

Host: cross-shard H32 combines (pre: batch-low bits of x, feature-high
bits of w; post: the mirror factors on the gathered output), layout
prep, bias.
"""
import sys
sys.path.insert(0, '/opt/trn_rl_repo')
import numpy as np
import ml_dtypes

B, IN, F = 4096, 2048, 4096
NCORES = 8
CS = IN // NCORES      # 256  per-core IN slice of x
FS = F // NCORES       # 512  per-core feature block of w
QMAX = 127.0
BF16 = ml_dtypes.bfloat16
FP16 = np.float16

_cache = {}


def _sylvester(n):
    h = np.array([[1.0]], dtype=np.float32)
    while h.shape[0] < n:
        h = np.block([[h, h], [h, -h]])
    return h


def _build():
    from concourse import bass, bacc, tile, mybir
    import concourse.bass_isa as bass_isa

    DT = mybir.dt.float32
    F16 = mybir.dt.float16
    I32 = mybir.dt.int32
    A = mybir.AluOpType
    npf16 = mybir.dt.np(F16)

    nc = bacc.Bacc("TRN2", target_bir_lowering=False, debug=False,
                   num_devices=NCORES)

    # host-prepped inputs (fp16)
    xh = nc.dram_tensor("xh", [128, 8192], F16, kind="ExternalInput")
    nk = nc.dram_tensor("nk", [128, 8192], F16, kind="ExternalInput")
    wh = nc.dram_tensor("wh", [512, 2048], F16, kind="ExternalInput")
    mk = nc.dram_tensor("mk", [512, 2048], F16, kind="ExternalInput")
    out = nc.dram_tensor("out", [512, 4096], F16, kind="ExternalOutput")

    dum_i = nc.dram_tensor("dum_i", [1, 1], DT)
    dum_o = nc.dram_tensor("dum_o", [1, 1], DT, addr_space="Shared")
    sx_i = nc.dram_tensor("sx_i", [1, 1], DT)
    sx_o = nc.dram_tensor("sx_o", [1, 1], DT, addr_space="Shared")
    sw_i = nc.dram_tensor("sw_i", [1, 1], DT)
    sw_o = nc.dram_tensor("sw_o", [1, 1], DT, addr_space="Shared")
    # batch-quarter AllGather payloads
    xqc = [nc.dram_tensor(f"xqc{q}", [256, 1024], F16) for q in range(4)]
    xqg = [nc.dram_tensor(f"xqg{q}", [2048, 1024], F16,
                          addr_space="Shared") for q in range(4)]

    h128h_d = nc.inline_tensor(_sylvester(128).astype(npf16), name="h128h")
    rg = [list(range(NCORES))]

    with tile.TileContext(nc) as tc:
      with tc.tile_pool(name="consts", bufs=1) as cpool, \
           tc.tile_pool(name="big", bufs=1) as bigp, \
           tc.tile_pool(name="qT", bufs=1) as qTp, \
           tc.tile_pool(name="qsc", bufs=1) as qsc:
        h128h = cpool.tile([128, 128], F16)
        nc.sync.dma_start(h128h[:], h128h_d[:])

        def scale_finish(tag, cc_out):
            # cc_out already holds the global QMAX/s (AllReduce-min of the
            # per-core reciprocals); just fetch + broadcast.
            sg = qsc.tile([1, 1], DT, tag=f"sg{tag}", name=f"sg{tag}")
            nc.sync.dma_start(sg[0:1, :], cc_out[:])
            rb = qsc.tile([128, 1], DT, tag=f"rb{tag}", name=f"rb{tag}")
            nc.gpsimd.partition_broadcast(rb[:, 0:1], sg[0:1, 0:1])
            return sg, rb

        with tc.tile_pool(name="fin", bufs=1) as fin, \
             tc.tile_pool(name="qtmp", bufs=2) as qtmp, \
             tc.tile_pool(name="fps", bufs=4, space="PSUM") as psp:

            # ---------- forward H128 (x) ----------
            xrB = bigp.tile([128, 8192], DT, tag="big1", name="xrB")
            amxp = qsc.tile([128, 16], DT, tag="amxp", name="amxp")
            for q in range(4):
                qsl = slice(q * 2048, (q + 1) * 2048)
                xb = fin.tile([128, 2048], F16, tag="xb", name="xb", bufs=2)
                nc.sync.dma_start(xb[:], xh[:, qsl])
                for jj in range(4):
                    j = q * 4 + jj
                    sl = slice(j * 512, (j + 1) * 512)
                    lsl = slice(jj * 512, (jj + 1) * 512)
                    ps = psp.tile([128, 512], DT, tag="fps", name="fpst")
                    nc.tensor.matmul(ps[:], h128h[:], xb[:, lsl],
                                     start=True, stop=True)
                    nc.scalar.copy(xrB[:, sl], ps[:])
                    nc.vector.tensor_reduce(
                        amxp[:, j:j + 1], ps[:], axis=mybir.AxisListType.X,
                        op=A.max, apply_absolute_value=True)
            amx = qsc.tile([128, 1], DT, tag="amx", name="amx")
            nc.vector.tensor_reduce(amx[:], amxp[:],
                                    axis=mybir.AxisListType.X, op=A.max,
                                    apply_absolute_value=True)
            rdx = qsc.tile([128, 1], DT, tag="rdx", name="rdx")
            nc.gpsimd.partition_all_reduce(
                rdx[:], amx[:], channels=128,
                reduce_op=bass_isa.ReduceOp.absmax)
            # local QMAX/absmax; AllReduce-min == QMAX/global_absmax since
            # x -> QMAX/x is monotone decreasing (exactly, in fp too)
            rxl = qsc.tile([1, 1], DT, tag="rxl", name="rxl")
            nc.vector.reciprocal(rxl[0:1, :], rdx[0:1, 0:1])
            nc.vector.tensor_scalar_mul(rxl[0:1, :], rxl[0:1, :], QMAX)
            nc.sync.dma_start(sx_i[:], rxl[0:1, 0:1])
            nc.gpsimd.collective_compute(
                "AllReduce", A.min, replica_groups=rg,
                ins=[sx_i.ap().opt()], outs=[sx_o.ap().opt()])

            # ---------- forward H128 (w) ----------
            wrB = bigp.tile([128, 8192], DT, tag="big2", name="wrB")
            amwp = qsc.tile([128, 16], DT, tag="amwp", name="amwp")
            for u in range(4):
                rsl = slice(u * 128, (u + 1) * 128)
                wb = fin.tile([128, 2048], F16, tag="wb", name="wb", bufs=2)
                nc.sync.dma_start(wb[:], wh[rsl, :])
                for j in range(4):
                    sl = slice(j * 512, (j + 1) * 512)
                    ps = psp.tile([128, 512], DT, tag="fps", name="fpsw")
                    nc.tensor.matmul(ps[:], h128h[:], wb[:, sl],
                                     start=True, stop=True)
                    osl = slice(u * 2048 + j * 512, u * 2048 + (j + 1) * 512)
                    nc.scalar.copy(wrB[:, osl], ps[:])
                    nc.vector.tensor_reduce(
                        amwp[:, u * 4 + j:u * 4 + j + 1], ps[:],
                        axis=mybir.AxisListType.X, op=A.max,
                        apply_absolute_value=True)
            amw = qsc.tile([128, 1], DT, tag="amw", name="amw")
            nc.vector.tensor_reduce(amw[:], amwp[:],
                                    axis=mybir.AxisListType.X, op=A.max,
                                    apply_absolute_value=True)
            rdw = qsc.tile([128, 1], DT, tag="rdw", name="rdw")
            nc.gpsimd.partition_all_reduce(
                rdw[:], amw[:], channels=128,
                reduce_op=bass_isa.ReduceOp.absmax)
            rwl = qsc.tile([1, 1], DT, tag="rwl", name="rwl")
            nc.vector.reciprocal(rwl[0:1, :], rdw[0:1, 0:1])
            nc.vector.tensor_scalar_mul(rwl[0:1, :], rwl[0:1, :], QMAX)
            nc.sync.dma_start(sw_i[:], rwl[0:1, 0:1])
            nc.gpsimd.collective_compute(
                "AllReduce", A.min, replica_groups=rg,
                ins=[sw_i.ap().opt()], outs=[sw_o.ap().opt()])

            def quant_chunk(big, rb, noise_ap, ch, tpool, ppool, ptag):
                """stt + cast one 1024-col chunk -> fp16 int-valued tile."""
                sl = slice(ch * 1024, (ch + 1) * 1024)
                nz = tpool.tile([128, 1024], F16, tag="nz", name="nzt",
                                bufs=2)
                nc.sync.dma_start(nz[:], noise_ap(ch))
                qi = tpool.tile([128, 1024], I32, tag="qi", name="qit",
                                bufs=2)
                nc.vector.scalar_tensor_tensor(
                    qi[:], big[:, sl], rb[:, 0:1], nz[:],
                    op0=A.mult, op1=A.add)
                qc = tpool.tile([128, 1024], F16, tag="qc", name="qct",
                                bufs=2)
                if ch % 2 == 0:
                    nc.vector.tensor_copy(qc[:], qi[:])
                else:
                    nc.scalar.copy(qc[:], qi[:])
                return qc

            # ---------- x quant + fused fold/transpose + 4-way AG ------
            sgx, rbx = scale_finish("x", sx_o)
            xqT = [qTp.tile([128, 4096], F16, tag=f"xqT{h}",
                            name=f"xqT{h}") for h in range(2)]

            def x_quarter(quarter, tpool, ppool, ptag, pbufs):
                for ch in range(quarter * 2, quarter * 2 + 2):
                    qc = quant_chunk(
                        xrB, rbx,
                        lambda c: nk[:, c * 1024:(c + 1) * 1024], ch,
                        tpool, ppool, ptag)
                    pstt = [ppool.tile([128, 512], DT, tag=ptag,
                                       name=f"pstx{h}", bufs=pbufs)
                            for h in range(2)]
                    for p in range(8):
                        r, h = p // 2, p % 2
                        nc.tensor.matmul(
                            pstt[h][:, r * 128:(r + 1) * 128],
                            qc[:, p * 128:(p + 1) * 128], h128h[:],
                            start=True, stop=True)
                    osl = slice(ch * 512, (ch + 1) * 512)
                    nc.vector.tensor_copy(xqT[0][:, osl], pstt[0][:])
                    nc.scalar.copy(xqT[1][:, osl], pstt[1][:])
                csl = slice(quarter * 1024, (quarter + 1) * 1024)
                nc.sync.dma_start(xqc[quarter][0:128, :], xqT[0][:, csl])
                nc.sync.dma_start(xqc[quarter][128:256, :], xqT[1][:, csl])
                nc.gpsimd.collective_compute(
                    "AllGather", A.bypass, replica_groups=rg,
                    ins=[xqc[quarter].ap().opt()],
                    outs=[xqg[quarter].ap().opt()])

            wblk = qTp.tile([128, 8192], F16, tag="wblk", name="wblk")
            wblk4 = wblk[:].rearrange("p (t s q) -> p t s q", t=16, s=4)

            def w_chunk(ch, rbw):
                u, hv = ch // 2, ch % 2
                qc = quant_chunk(
                    wrB, rbw,
                    lambda c: mk[(c // 2) * 128:(c // 2 + 1) * 128,
                                 (c % 2) * 1024:(c % 2 + 1) * 1024], ch,
                    qtmp, psp, "pst")
                for q2 in range(2):
                    pst = psp.tile([128, 512], DT, tag="pst", name="pstw")
                    for r in range(4):
                        p = q2 * 4 + r
                        nc.tensor.matmul(
                            pst[:, r * 128:(r + 1) * 128],
                            qc[:, p * 128:(p + 1) * 128], h128h[:],
                            start=True, stop=True)
                    v0 = hv * 8 + q2 * 4
                    dst = wblk4[:, v0:v0 + 4, u, :]
                    src = pst[:].rearrange("p (r q) -> p r q", r=4)
                    if ch % 2 == 0:
                        nc.vector.tensor_copy(dst, src)
                    else:
                        nc.scalar.copy(dst, src)

            # interleave: x quarters gate the AllGather train (critical);
            # w chunks fill the gaps, i-half 0 first so early GEMM k-tiles
            # are ready.  Quarters 2-3 are emitted in the GEMM scope so the
            # PSUM pool handoff doesn't delay the GEMM start.
            x_quarter(0, qtmp, psp, "pst", 4)
            sgw, rbw = scale_finish("w", sw_o)
            for ch in (0, 2, 4, 6):
                w_chunk(ch, rbw)
            x_quarter(1, qtmp, psp, "pst", 4)
            for ch in (1, 3, 5, 7):
                w_chunk(ch, rbw)

            # alpha = sx*sw/(QMAX^2 * 2^24) = 1/(rbx*rbw*2^24)
            alp = qsc.tile([1, 1], DT, tag="alp", name="alp")
            nc.vector.tensor_tensor(alp[0:1, 0:1], sgx[0:1, 0:1],
                                    sgw[0:1, 0:1], op=A.mult)
            nc.vector.tensor_scalar_mul(alp[0:1, 0:1], alp[0:1, 0:1],
                                        float(1 << 24))
            al = qsc.tile([1, 1], DT, tag="al", name="al")
            nc.vector.reciprocal(al[0:1, 0:1], alp[0:1, 0:1])
            alb = qsc.tile([128, 1], DT, tag="alb", name="alb")
            nc.gpsimd.partition_broadcast(alb[:, 0:1], al[0:1, 0:1])

        # ---------- GEMM (output is final up to alpha) ----------------
        with tc.tile_pool(name="gem", bufs=8) as gem, \
             tc.tile_pool(name="gps", bufs=8, space="PSUM") as gps:
            wblk_g = wblk  # keep referenced
            for g in range(4):
                if g == 1:
                    # quarters 2-3 fold+AG between groups: their AGs land
                    # well before groups 2-3 need them, and the fold
                    # matmuls slot into the PE stream after group 0
                    x_quarter(2, gem, gps, "gp", 8)
                    x_quarter(3, gem, gps, "gp", 8)
                psY = [gps.tile([128, 512], DT, tag="gp",
                                name=f"gpt{g}_{j}", bufs=8)
                       for j in range(8)]
                for t in range(16):
                    xt = gem.tile([128, 1024], F16, tag="xt", name="xtt")
                    nc.sync.dma_start(
                        xt[:], xqg[g][(t // 2) * 256 + (t % 2) * 128:
                                      (t // 2) * 256 + (t % 2) * 128 + 128,
                                      :])
                    for s in range(4):
                        for hb in range(2):
                            nc.tensor.matmul(
                                psY[s * 2 + hb][:],
                                wblk_g[:, t * 512 + s * 128:
                                       t * 512 + (s + 1) * 128],
                                xt[:, hb * 512:(hb + 1) * 512],
                                start=(t == 0), stop=(t == 15))
                for s in range(4):
                    for hb in range(2):
                        ot = gem.tile([128, 512], F16, tag="ot",
                                      name="ott", bufs=4)
                        nc.vector.tensor_scalar(
                            ot[:], psY[s * 2 + hb][:], alb[:, 0:1], None,
                            op0=A.mult)
                        nc.sync.dma_start(
                            out[s * 128:(s + 1) * 128,
                                g * 1024 + hb * 512:
                                g * 1024 + (hb + 1) * 512], ot[:])
    nc.compile()
    return nc


def make_in_maps(inputs):
    H32 = _sylvester(32)
    x = np.asarray(inputs["inputs"], np.float32)
    w = np.asarray(inputs["kernel"], np.float32)
    nxp = (0.5 - np.asarray(inputs["noise_x"], np.float32))
    nwp = (0.5 - np.asarray(inputs["noise_w"], np.float32))

    # host cross-shard combines (H32 factors)
    xhf = np.einsum('st,bti->bsi', H32, x.reshape(128, 32, IN))
    whf = np.einsum('st,itp->isp', H32, w.reshape(IN, 32, 128))
    nx3 = nxp.reshape(128, 32, IN)

    in_maps = []
    for k in range(NCORES):
        xs = np.ascontiguousarray(xhf[:, :, k * CS:(k + 1) * CS]) \
               .reshape(128, 8192).astype(FP16)
        nks = np.ascontiguousarray(nx3[:, :, k * CS:(k + 1) * CS]) \
                .reshape(128, 8192).astype(FP16)
        ws = np.ascontiguousarray(whf[:, 4 * k:4 * k + 4, :]
                                  .transpose(1, 2, 0)) \
               .reshape(512, IN).astype(FP16)
        mks = np.ascontiguousarray(
            nwp[:, k * FS:(k + 1) * FS].T).astype(FP16)
        in_maps.append({"xh": xs, "nk": nks, "wh": ws, "mk": mks})
    return in_maps


def kernel(**inputs):
    from concourse.bass_utils import run_bass_kernel_spmd

    if "nc" not in _cache:
        _cache["nc"] = _build()
    nc = _cache["nc"]

    bias = np.asarray(inputs["bias"], np.float32)
    in_maps = make_in_maps(inputs)

    res = run_bass_kernel_spmd(nc, in_maps, list(range(NCORES)))

    # host unshard: H32 mirror factors over feature-blocks and batch-low
    H32 = _sylvester(32)
    V = np.stack([r["out"].astype(np.float32) for r in res.results])
    V = V.reshape(NCORES, 4, 128, 32, 128)        # [a, u, q, b2, b1]
    V = V.reshape(32, 128, 32, 128)               # [g=(a,u), q, b2, b1]
    V = np.einsum('st,tqbj->sqbj', H32, V)        # H32 over feature blocks
    V = np.einsum('cd,sqdj->sqcj', H32, V)        # H32 over batch-low
    y = V.transpose(3, 2, 0, 1).reshape(B, F)     # [b1, b2, g, q] -> [B, F]
    return (y + bias[None, :]).astype(np.float32)


# revision 30
# speedup vs baseline: 1.0931x; 1.0931x over previous
"""Trainium2 Bass kernel for quantized dense layer with Hadamard rotations.

Math (reference): y = (H2 @ (sq(H2@x) @ sq(w@H1)) @ H1)/4096 + bias,
sq() = per-tensor symmetric int8 stochastic quantization.

Sharding (8 cores), per the data-parallel + per-shard-Hadamard hint:
Sylvester Hadamards factor as Kronecker products; the cross-shard H32
factors are folded into the host-side shard/unshard combines, while
each core applies the per-shard H128 factors on device.  Forward: fp16
operands, H128 PE matmuls with fp32 accumulation; global quant scales
via two 1-scalar AllReduces; stochastic rounding via the fp32->int32
round-to-nearest cast with host-precomputed 0.5-noise.  The two
inverse per-shard H128 factors are folded into the quantized operands
before the GEMM (H_B (xq wq) H_F = (H_B xq)(wq H_F)) as fused
fold+transpose matmuls, so the fp16 GEMM output is final up to the
alpha scale.  Quantized activations are exchanged with a 4-way
batch-quarter AllGather pipelined against the GEMM groups; w arrives
feature-sharded so no AllToAll is needed.

Host: cross-shard H32 combines (pre: batch-low bits of x, feature-high
bits of w; post: the mirror factors on the gathered output), layout
prep, bias.
"""
import sys
sys.path.insert(0, '/opt/trn_rl_repo')
import numpy as np
import ml_dtypes

B, IN, F = 4096, 2048, 4096
NCORES = 8
CS = IN // NCORES      # 256  per-core IN slice of x
FS = F // NCORES       # 512  per-core feature block of w
QMAX = 127.0
BF16 = ml_dtypes.bfloat16
FP16 = np.float16

_cache = {}


def _sylvester(n):
    h = np.array([[1.0]], dtype=np.float32)
    while h.shape[0] < n:
        h = np.block([[h, h], [h, -h]])
    return h


def _build():
    from concourse import bass, bacc, tile, mybir
    import concourse.bass_isa as bass_isa

    DT = mybir.dt.float32
    F16 = mybir.dt.float16
    I32 = mybir.dt.int32
    A = mybir.AluOpType
    npf16 = mybir.dt.np(F16)

    nc = bacc.Bacc("TRN2", target_bir_lowering=False, debug=False,
                   num_devices=NCORES)

    # host-prepped inputs (fp16)
    xh = nc.dram_tensor("xh", [128, 8192], F16, kind="ExternalInput")
    nk = nc.dram_tensor("nk", [128, 8192], F16, kind="ExternalInput")
    wh = nc.dram_tensor("wh", [512, 2048], F16, kind="ExternalInput")
    mk = nc.dram_tensor("mk", [512, 2048], F16, kind="ExternalInput")
    out = nc.dram_tensor("out", [512, 4096], F16, kind="ExternalOutput")

    dum_i = nc.dram_tensor("dum_i", [1, 1], DT)
    dum_o = nc.dram_tensor("dum_o", [1, 1], DT, addr_space="Shared")
    sx_i = nc.dram_tensor("sx_i", [1, 1], DT)
    sx_o = nc.dram_tensor("sx_o", [1, 1], DT, addr_space="Shared")
    sw_i = nc.dram_tensor("sw_i", [1, 1], DT)
    sw_o = nc.dram_tensor("sw_o", [1, 1], DT, addr_space="Shared")
    # batch-quarter AllGather payloads
    xqc = [nc.dram_tensor(f"xqc{q}", [256, 1024], F16) for q in range(4)]
    xqg = [nc.dram_tensor(f"xqg{q}", [2048, 1024], F16,
                          addr_space="Shared") for q in range(4)]

    h128h_d = nc.inline_tensor(_sylvester(128).astype(npf16), name="h128h")
    rg = [list(range(NCORES))]

    with tile.TileContext(nc) as tc:
      with tc.tile_pool(name="consts", bufs=1) as cpool, \
           tc.tile_pool(name="big", bufs=1) as bigp, \
           tc.tile_pool(name="qT", bufs=1) as qTp, \
           tc.tile_pool(name="qsc", bufs=1) as qsc:
        h128h = cpool.tile([128, 128], F16)
        nc.sync.dma_start(h128h[:], h128h_d[:])

        def scale_finish(tag, cc_out):
            # cc_out already holds the global QMAX/s (AllReduce-min of the
            # per-core reciprocals); just fetch + broadcast.
            sg = qsc.tile([1, 1], DT, tag=f"sg{tag}", name=f"sg{tag}")
            nc.sync.dma_start(sg[0:1, :], cc_out[:])
            rb = qsc.tile([128, 1], DT, tag=f"rb{tag}", name=f"rb{tag}")
            nc.gpsimd.partition_broadcast(rb[:, 0:1], sg[0:1, 0:1])
            return sg, rb

        with tc.tile_pool(name="fin", bufs=1) as fin, \
             tc.tile_pool(name="qtmp", bufs=2) as qtmp, \
             tc.tile_pool(name="fps", bufs=4, space="PSUM") as psp:

            # ---------- forward H128 (x) ----------
            xrB = bigp.tile([128, 8192], DT, tag="big1", name="xrB")
            amxp = qsc.tile([128, 16], DT, tag="amxp", name="amxp")
            for q in range(4):
                qsl = slice(q * 2048, (q + 1) * 2048)
                xb = fin.tile([128, 2048], F16, tag="xb", name="xb", bufs=2)
                nc.sync.dma_start(xb[:], xh[:, qsl])
                for jj in range(4):
                    j = q * 4 + jj
                    sl = slice(j * 512, (j + 1) * 512)
                    lsl = slice(jj * 512, (jj + 1) * 512)
                    ps = psp.tile([128, 512], DT, tag="fps", name="fpst")
                    nc.tensor.matmul(ps[:], h128h[:], xb[:, lsl],
                                     start=True, stop=True)
                    nc.scalar.copy(xrB[:, sl], ps[:])
                    nc.vector.tensor_reduce(
                        amxp[:, j:j + 1], ps[:], axis=mybir.AxisListType.X,
                        op=A.max, apply_absolute_value=True)
            amx = qsc.tile([128, 1], DT, tag="amx", name="amx")
            nc.vector.tensor_reduce(amx[:], amxp[:],
                                    axis=mybir.AxisListType.X, op=A.max,
                                    apply_absolute_value=True)
            rdx = qsc.tile([128, 1], DT, tag="rdx", name="rdx")
            nc.gpsimd.partition_all_reduce(
                rdx[:], amx[:], channels=128,
                reduce_op=bass_isa.ReduceOp.absmax)
            # local QMAX/absmax; AllReduce-min == QMAX/global_absmax since
            # x -> QMAX/x is monotone decreasing (exactly, in fp too)
            rxl = qsc.tile([1, 1], DT, tag="rxl", name="rxl")
            nc.vector.reciprocal(rxl[0:1, :], rdx[0:1, 0:1])
            nc.vector.tensor_scalar_mul(rxl[0:1, :], rxl[0:1, :], QMAX)
            nc.sync.dma_start(sx_i[:], rxl[0:1, 0:1])
            nc.gpsimd.collective_compute(
                "AllReduce", A.min, replica_groups=rg,
                ins=[sx_i.ap().opt()], outs=[sx_o.ap().opt()])

            # ---------- forward H128 (w) ----------
            wrB = bigp.tile([128, 8192], DT, tag="big2", name="wrB")
            amwp = qsc.tile([128, 16], DT, tag="amwp", name="amwp")
            for u in range(4):
                rsl = slice(u * 128, (u + 1) * 128)
                wb = fin.tile([128, 2048], F16, tag="wb", name="wb", bufs=2)
                nc.sync.dma_start(wb[:], wh[rsl, :])
                for j in range(4):
                    sl = slice(j * 512, (j + 1) * 512)
                    ps = psp.tile([128, 512], DT, tag="fps", name="fpsw")
                    nc.tensor.matmul(ps[:], h128h[:], wb[:, sl],
                                     start=True, stop=True)
                    osl = slice(u * 2048 + j * 512, u * 2048 + (j + 1) * 512)
                    nc.scalar.copy(wrB[:, osl], ps[:])
                    nc.vector.tensor_reduce(
                        amwp[:, u * 4 + j:u * 4 + j + 1], ps[:],
                        axis=mybir.AxisListType.X, op=A.max,
                        apply_absolute_value=True)
            amw = qsc.tile([128, 1], DT, tag="amw", name="amw")
            nc.vector.tensor_reduce(amw[:], amwp[:],
                                    axis=mybir.AxisListType.X, op=A.max,
                                    apply_absolute_value=True)
            rdw = qsc.tile([128, 1], DT, tag="rdw", name="rdw")
            nc.gpsimd.partition_all_reduce(
                rdw[:], amw[:], channels=128,
                reduce_op=bass_isa.ReduceOp.absmax)
            rwl = qsc.tile([1, 1], DT, tag="rwl", name="rwl")
            nc.vector.reciprocal(rwl[0:1, :], rdw[0:1, 0:1])
            nc.vector.tensor_scalar_mul(rwl[0:1, :], rwl[0:1, :], QMAX)
            nc.sync.dma_start(sw_i[:], rwl[0:1, 0:1])
            nc.gpsimd.collective_compute(
                "AllReduce", A.min, replica_groups=rg,
                ins=[sw_i.ap().opt()], outs=[sw_o.ap().opt()])

            def quant_chunk(big, rb, noise_ap, ch, tpool, ppool, ptag):
                """stt + cast one 1024-col chunk -> fp16 int-valued tile."""
                sl = slice(ch * 1024, (ch + 1) * 1024)
                nz = tpool.tile([128, 1024], F16, tag="nz", name="nzt",
                                bufs=2)
                nc.sync.dma_start(nz[:], noise_ap(ch))
                qi = tpool.tile([128, 1024], I32, tag="qi", name="qit",
                                bufs=2)
                nc.vector.scalar_tensor_tensor(
                    qi[:], big[:, sl], rb[:, 0:1], nz[:],
                    op0=A.mult, op1=A.add)
                qc = tpool.tile([128, 1024], F16, tag="qc", name="qct",
                                bufs=2)
                if ch % 2 == 0:
                    nc.vector.tensor_copy(qc[:], qi[:])
                else:
                    nc.scalar.copy(qc[:], qi[:])
                return qc

            # ---------- x quant + fused fold/transpose + 4-way AG ------
            sgx, rbx = scale_finish("x", sx_o)
            xqT = [qTp.tile([128, 4096], F16, tag=f"xqT{h}",
                            name=f"xqT{h}") for h in range(2)]

            def x_quarter(quarter, tpool, ppool, ptag, pbufs):
                for ch in range(quarter * 2, quarter * 2 + 2):
                    qc = quant_chunk(
                        xrB, rbx,
                        lambda c: nk[:, c * 1024:(c + 1) * 1024], ch,
                        tpool, ppool, ptag)
                    pstt = [ppool.tile([128, 512], DT, tag=ptag,
                                       name=f"pstx{h}", bufs=pbufs)
                            for h in range(2)]
                    for p in range(8):
                        r, h = p // 2, p % 2
                        nc.tensor.matmul(
                            pstt[h][:, r * 128:(r + 1) * 128],
                            qc[:, p * 128:(p + 1) * 128], h128h[:],
                            start=True, stop=True)
                    osl = slice(ch * 512, (ch + 1) * 512)
                    nc.vector.tensor_copy(xqT[0][:, osl], pstt[0][:])
                    nc.scalar.copy(xqT[1][:, osl], pstt[1][:])
                csl = slice(quarter * 1024, (quarter + 1) * 1024)
                nc.sync.dma_start(xqc[quarter][0:128, :], xqT[0][:, csl])
                nc.sync.dma_start(xqc[quarter][128:256, :], xqT[1][:, csl])
                nc.gpsimd.collective_compute(
                    "AllGather", A.bypass, replica_groups=rg,
                    ins=[xqc[quarter].ap().opt()],
                    outs=[xqg[quarter].ap().opt()])

            wblk = qTp.tile([128, 8192], F16, tag="wblk", name="wblk")
            wblk4 = wblk[:].rearrange("p (t s q) -> p t s q", t=16, s=4)

            def w_chunk(ch, rbw):
                u, hv = ch // 2, ch % 2
                qc = quant_chunk(
                    wrB, rbw,
                    lambda c: mk[(c // 2) * 128:(c // 2 + 1) * 128,
                                 (c % 2) * 1024:(c % 2 + 1) * 1024], ch,
                    qtmp, psp, "pst")
                for q2 in range(2):
                    pst = psp.tile([128, 512], DT, tag="pst", name="pstw")
                    for r in range(4):
                        p = q2 * 4 + r
                        nc.tensor.matmul(
                            pst[:, r * 128:(r + 1) * 128],
                            qc[:, p * 128:(p + 1) * 128], h128h[:],
                            start=True, stop=True)
                    v0 = hv * 8 + q2 * 4
                    dst = wblk4[:, v0:v0 + 4, u, :]
                    src = pst[:].rearrange("p (r q) -> p r q", r=4)
                    if ch % 2 == 0:
                        nc.vector.tensor_copy(dst, src)
                    else:
                        nc.scalar.copy(dst, src)

            # x quarters gate the AllGather train (critical path): emit all
            # four first so the AG triggers only depend on the x quant
            # chain.  w chunks follow (i-half 0 first, so early GEMM
            # k-tiles are ready sooner); the GEMM can't start before wblk
            # regardless.
            x_quarter(0, qtmp, psp, "pst", 4)
            sgw, rbw = scale_finish("w", sw_o)
            x_quarter(1, qtmp, psp, "pst", 4)
            x_quarter(2, qtmp, psp, "pst", 4)
            x_quarter(3, qtmp, psp, "pst", 4)
            for ch in (0, 2, 4, 6, 1, 3, 5, 7):
                w_chunk(ch, rbw)

            # alpha = sx*sw/(QMAX^2 * 2^24) = 1/(rbx*rbw*2^24)
            alp = qsc.tile([1, 1], DT, tag="alp", name="alp")
            nc.vector.tensor_tensor(alp[0:1, 0:1], sgx[0:1, 0:1],
                                    sgw[0:1, 0:1], op=A.mult)
            nc.vector.tensor_scalar_mul(alp[0:1, 0:1], alp[0:1, 0:1],
                                        float(1 << 24))
            al = qsc.tile([1, 1], DT, tag="al", name="al")
            nc.vector.reciprocal(al[0:1, 0:1], alp[0:1, 0:1])
            alb = qsc.tile([128, 1], DT, tag="alb", name="alb")
            nc.gpsimd.partition_broadcast(alb[:, 0:1], al[0:1, 0:1])

        # ---------- GEMM (output is final up to alpha) ----------------
        with tc.tile_pool(name="gem", bufs=8) as gem, \
             tc.tile_pool(name="gps", bufs=8, space="PSUM") as gps:
            wblk_g = wblk  # keep referenced
            for g in range(4):
                psY = [gps.tile([128, 512], DT, tag="gp",
                                name=f"gpt{g}_{j}", bufs=8)
                       for j in range(8)]
                for t in range(16):
                    xt = gem.tile([128, 1024], F16, tag="xt", name="xtt")
                    nc.sync.dma_start(
                        xt[:], xqg[g][(t // 2) * 256 + (t % 2) * 128:
                                      (t // 2) * 256 + (t % 2) * 128 + 128,
                                      :])
                    for s in range(4):
                        for hb in range(2):
                            nc.tensor.matmul(
                                psY[s * 2 + hb][:],
                                wblk_g[:, t * 512 + s * 128:
                                       t * 512 + (s + 1) * 128],
                                xt[:, hb * 512:(hb + 1) * 512],
                                start=(t == 0), stop=(t == 15))
                for s in range(4):
                    for hb in range(2):
                        ot = gem.tile([128, 512], F16, tag="ot",
                                      name="ott", bufs=4)
                        nc.vector.tensor_scalar(
                            ot[:], psY[s * 2 + hb][:], alb[:, 0:1], None,
                            op0=A.mult)
                        nc.sync.dma_start(
                            out[s * 128:(s + 1) * 128,
                                g * 1024 + hb * 512:
                                g * 1024 + (hb + 1) * 512], ot[:])
    nc.compile()
    return nc


def make_in_maps(inputs):
    H32 = _sylvester(32)
    x = np.asarray(inputs["inputs"], np.float32)
    w = np.asarray(inputs["kernel"], np.float32)
    nxp = (0.5 - np.asarray(inputs["noise_x"], np.float32))
    nwp = (0.5 - np.asarray(inputs["noise_w"], np.float32))

    # host cross-shard combines (H32 factors)
    xhf = np.einsum('st,bti->bsi', H32, x.reshape(128, 32, IN))
    whf = np.einsum('st,itp->isp', H32, w.reshape(IN, 32, 128))
    nx3 = nxp.reshape(128, 32, IN)

    in_maps = []
    for k in range(NCORES):
        xs = np.ascontiguousarray(xhf[:, :, k * CS:(k + 1) * CS]) \
               .reshape(128, 8192).astype(FP16)
        nks = np.ascontiguousarray(nx3[:, :, k * CS:(k + 1) * CS]) \
                .reshape(128, 8192).astype(FP16)
        ws = np.ascontiguousarray(whf[:, 4 * k:4 * k + 4, :]
                                  .transpose(1, 2, 0)) \
               .reshape(512, IN).astype(FP16)
        mks = np.ascontiguousarray(
            nwp[:, k * FS:(k + 1) * FS].T).astype(FP16)
        in_maps.append({"xh": xs, "nk": nks, "wh": ws, "mk": mks})
    return in_maps


def kernel(**inputs):
    from concourse.bass_utils import run_bass_kernel_spmd

    if "nc" not in _cache:
        _cache["nc"] = _build()
    nc = _cache["nc"]

    bias = np.asarray(inputs["bias"], np.float32)
    in_maps = make_in_maps(inputs)

    res = run_bass_kernel_spmd(nc, in_maps, list(range(NCORES)))

    # host unshard: H32 mirror factors over feature-blocks and batch-low
    H32 = _sylvester(32)
    V = np.stack([r["out"].astype(np.float32) for r in res.results])
    V = V.reshape(NCORES, 4, 128, 32, 128)        # [a, u, q, b2, b1]
    V = V.reshape(32, 128, 32, 128)               # [g=(a,u), q, b2, b1]
    V = np.einsum('st,tqbj->sqbj', H32, V)        # H32 over feature blocks
    V = np.einsum('cd,sqdj->sqcj', H32, V)        # H32 over batch-low
    y = V.transpose(3, 2, 0, 1).reshape(B, F)     # [b1, b2, g, q] -> [B, F]
    return (y + bias[None, :]).astype(np.float32)


# revision 32
# speedup vs baseline: 1.1236x; 1.0279x over previous
"""Trainium2 Bass kernel for quantized dense layer with Hadamard rotations.

Math (reference): y = (H2 @ (sq(H2@x) @ sq(w@H1)) @ H1)/4096 + bias,
sq() = per-tensor symmetric int8 stochastic quantization.

Sharding (8 cores), per the data-parallel + per-shard-Hadamard hint:
Sylvester Hadamards factor as Kronecker products; the cross-shard H32
factors are folded into the host-side shard/unshard combines, while
each core applies the per-shard H128 factors on device.  Forward: fp16
operands, H128 PE matmuls with fp32 accumulation; global quant scales
via two 1-scalar AllReduces; stochastic rounding via the fp32->int32
round-to-nearest cast with host-precomputed 0.5-noise.  The two
inverse per-shard H128 factors are folded into the quantized operands
before the GEMM (H_B (xq wq) H_F = (H_B xq)(wq H_F)) as fused
fold+transpose matmuls, so the fp16 GEMM output is final up to the
alpha scale.  Quantized activations are exchanged with a 4-way
batch-quarter AllGather pipelined against the GEMM groups; w arrives
feature-sharded so no AllToAll is needed.

Host: cross-shard H32 combines (pre: batch-low bits of x, feature-high
bits of w; post: the mirror factors on the gathered output), layout
prep, bias.
"""
import sys
sys.path.insert(0, '/opt/trn_rl_repo')
import numpy as np
import ml_dtypes

B, IN, F = 4096, 2048, 4096
NCORES = 8
CS = IN // NCORES      # 256  per-core IN slice of x
FS = F // NCORES       # 512  per-core feature block of w
QMAX = 127.0
BF16 = ml_dtypes.bfloat16
FP16 = np.float16

_cache = {}


def _sylvester(n):
    h = np.array([[1.0]], dtype=np.float32)
    while h.shape[0] < n:
        h = np.block([[h, h], [h, -h]])
    return h


def _build():
    from concourse import bass, bacc, tile, mybir
    import concourse.bass_isa as bass_isa

    DT = mybir.dt.float32
    F16 = mybir.dt.float16
    I32 = mybir.dt.int32
    A = mybir.AluOpType
    npf16 = mybir.dt.np(F16)

    nc = bacc.Bacc("TRN2", target_bir_lowering=False, debug=False,
                   num_devices=NCORES)

    # host-prepped inputs (fp16)
    xh = nc.dram_tensor("xh", [128, 8192], F16, kind="ExternalInput")
    nk = nc.dram_tensor("nk", [128, 8192], F16, kind="ExternalInput")
    wh = nc.dram_tensor("wh", [512, 2048], F16, kind="ExternalInput")
    mk = nc.dram_tensor("mk", [512, 2048], F16, kind="ExternalInput")
    out = nc.dram_tensor("out", [512, 4096], F16, kind="ExternalOutput")

    dum_i = nc.dram_tensor("dum_i", [1, 1], DT)
    dum_o = nc.dram_tensor("dum_o", [1, 1], DT, addr_space="Shared")
    sx_i = nc.dram_tensor("sx_i", [1, 1], DT)
    sx_o = nc.dram_tensor("sx_o", [1, 1], DT, addr_space="Shared")
    sw_i = nc.dram_tensor("sw_i", [1, 1], DT)
    sw_o = nc.dram_tensor("sw_o", [1, 1], DT, addr_space="Shared")
    # batch-quarter AllGather payloads
    xqc = [nc.dram_tensor(f"xqc{q}", [256, 1024], F16) for q in range(4)]
    xqg = [nc.dram_tensor(f"xqg{q}", [2048, 1024], F16,
                          addr_space="Shared") for q in range(4)]

    h128h_d = nc.inline_tensor(_sylvester(128).astype(npf16), name="h128h")
    rg = [list(range(NCORES))]

    with tile.TileContext(nc) as tc:
      with tc.tile_pool(name="consts", bufs=1) as cpool, \
           tc.tile_pool(name="big", bufs=1) as bigp, \
           tc.tile_pool(name="qT", bufs=1) as qTp, \
           tc.tile_pool(name="qsc", bufs=1) as qsc:
        h128h = cpool.tile([128, 128], F16)
        nc.sync.dma_start(h128h[:], h128h_d[:])

        def scale_finish(tag, cc_out):
            # cc_out already holds the global QMAX/s (AllReduce-min of the
            # per-core reciprocals); just fetch + broadcast.
            sg = qsc.tile([1, 1], DT, tag=f"sg{tag}", name=f"sg{tag}")
            nc.sync.dma_start(sg[0:1, :], cc_out[:])
            rb = qsc.tile([128, 1], DT, tag=f"rb{tag}", name=f"rb{tag}")
            nc.gpsimd.partition_broadcast(rb[:, 0:1], sg[0:1, 0:1])
            return sg, rb

        with tc.tile_pool(name="fin", bufs=1) as fin, \
             tc.tile_pool(name="qtmp", bufs=2) as qtmp, \
             tc.tile_pool(name="fps", bufs=4, space="PSUM") as psp:

            # ---------- forward H128 (x) ----------
            xrB = bigp.tile([128, 8192], DT, tag="big1", name="xrB")
            amxp = qsc.tile([128, 16], DT, tag="amxp", name="amxp")
            for q in range(4):
                qsl = slice(q * 2048, (q + 1) * 2048)
                xb = fin.tile([128, 2048], F16, tag="xb", name="xb", bufs=2)
                nc.sync.dma_start(xb[:], xh[:, qsl])
                for jj in range(4):
                    j = q * 4 + jj
                    sl = slice(j * 512, (j + 1) * 512)
                    lsl = slice(jj * 512, (jj + 1) * 512)
                    ps = psp.tile([128, 512], DT, tag="fps", name="fpst")
                    nc.tensor.matmul(ps[:], h128h[:], xb[:, lsl],
                                     start=True, stop=True)
                    nc.scalar.copy(xrB[:, sl], ps[:])
                    nc.vector.tensor_reduce(
                        amxp[:, j:j + 1], ps[:], axis=mybir.AxisListType.X,
                        op=A.max, apply_absolute_value=True)
            amx = qsc.tile([128, 1], DT, tag="amx", name="amx")
            nc.vector.tensor_reduce(amx[:], amxp[:],
                                    axis=mybir.AxisListType.X, op=A.max,
                                    apply_absolute_value=True)
            rdx = qsc.tile([128, 1], DT, tag="rdx", name="rdx")
            nc.gpsimd.partition_all_reduce(
                rdx[:], amx[:], channels=128,
                reduce_op=bass_isa.ReduceOp.absmax)
            # local QMAX/absmax; AllReduce-min == QMAX/global_absmax since
            # x -> QMAX/x is monotone decreasing (exactly, in fp too)
            rxl = qsc.tile([1, 1], DT, tag="rxl", name="rxl")
            nc.vector.reciprocal(rxl[0:1, :], rdx[0:1, 0:1])
            nc.vector.tensor_scalar_mul(rxl[0:1, :], rxl[0:1, :], QMAX)
            nc.sync.dma_start(sx_i[:], rxl[0:1, 0:1])
            nc.gpsimd.collective_compute(
                "AllReduce", A.min, replica_groups=rg,
                ins=[sx_i.ap().opt()], outs=[sx_o.ap().opt()])

            # ---------- forward H128 (w) ----------
            wrB = bigp.tile([128, 8192], DT, tag="big2", name="wrB")
            amwp = qsc.tile([128, 16], DT, tag="amwp", name="amwp")
            for u in range(4):
                rsl = slice(u * 128, (u + 1) * 128)
                wb = fin.tile([128, 2048], F16, tag="wb", name="wb", bufs=2)
                nc.sync.dma_start(wb[:], wh[rsl, :])
                for j in range(4):
                    sl = slice(j * 512, (j + 1) * 512)
                    ps = psp.tile([128, 512], DT, tag="fps", name="fpsw")
                    nc.tensor.matmul(ps[:], h128h[:], wb[:, sl],
                                     start=True, stop=True)
                    osl = slice(u * 2048 + j * 512, u * 2048 + (j + 1) * 512)
                    nc.scalar.copy(wrB[:, osl], ps[:])
                    nc.vector.tensor_reduce(
                        amwp[:, u * 4 + j:u * 4 + j + 1], ps[:],
                        axis=mybir.AxisListType.X, op=A.max,
                        apply_absolute_value=True)
            amw = qsc.tile([128, 1], DT, tag="amw", name="amw")
            nc.vector.tensor_reduce(amw[:], amwp[:],
                                    axis=mybir.AxisListType.X, op=A.max,
                                    apply_absolute_value=True)
            rdw = qsc.tile([128, 1], DT, tag="rdw", name="rdw")
            nc.gpsimd.partition_all_reduce(
                rdw[:], amw[:], channels=128,
                reduce_op=bass_isa.ReduceOp.absmax)
            rwl = qsc.tile([1, 1], DT, tag="rwl", name="rwl")
            nc.vector.reciprocal(rwl[0:1, :], rdw[0:1, 0:1])
            nc.vector.tensor_scalar_mul(rwl[0:1, :], rwl[0:1, :], QMAX)
            nc.sync.dma_start(sw_i[:], rwl[0:1, 0:1])
            nc.gpsimd.collective_compute(
                "AllReduce", A.min, replica_groups=rg,
                ins=[sw_i.ap().opt()], outs=[sw_o.ap().opt()])

            def quant_chunk(big, rb, noise_ap, ch, tpool, ppool, ptag):
                """stt + cast one 1024-col chunk -> fp16 int-valued tile."""
                sl = slice(ch * 1024, (ch + 1) * 1024)
                nz = tpool.tile([128, 1024], F16, tag="nz", name="nzt",
                                bufs=2)
                nc.sync.dma_start(nz[:], noise_ap(ch))
                qi = tpool.tile([128, 1024], I32, tag="qi", name="qit",
                                bufs=2)
                nc.vector.scalar_tensor_tensor(
                    qi[:], big[:, sl], rb[:, 0:1], nz[:],
                    op0=A.mult, op1=A.add)
                qc = tpool.tile([128, 1024], F16, tag="qc", name="qct",
                                bufs=2)
                nc.scalar.copy(qc[:], qi[:])
                return qc

            # ---------- x quant + fused fold/transpose + 4-way AG ------
            sgx, rbx = scale_finish("x", sx_o)
            xqT = [qTp.tile([128, 4096], F16, tag=f"xqT{h}",
                            name=f"xqT{h}") for h in range(2)]

            def x_quarter(quarter, tpool, ppool, ptag, pbufs):
                for ch in range(quarter * 2, quarter * 2 + 2):
                    qc = quant_chunk(
                        xrB, rbx,
                        lambda c: nk[:, c * 1024:(c + 1) * 1024], ch,
                        tpool, ppool, ptag)
                    pstt = [ppool.tile([128, 512], DT, tag=ptag,
                                       name=f"pstx{h}", bufs=pbufs)
                            for h in range(2)]
                    for p in range(8):
                        r, h = p // 2, p % 2
                        nc.tensor.matmul(
                            pstt[h][:, r * 128:(r + 1) * 128],
                            qc[:, p * 128:(p + 1) * 128], h128h[:],
                            start=True, stop=True)
                    osl = slice(ch * 512, (ch + 1) * 512)
                    nc.vector.tensor_copy(xqT[0][:, osl], pstt[0][:])
                    nc.scalar.copy(xqT[1][:, osl], pstt[1][:])
                csl = slice(quarter * 1024, (quarter + 1) * 1024)
                nc.sync.dma_start(xqc[quarter][0:128, :], xqT[0][:, csl])
                nc.sync.dma_start(xqc[quarter][128:256, :], xqT[1][:, csl])
                nc.gpsimd.collective_compute(
                    "AllGather", A.bypass, replica_groups=rg,
                    ins=[xqc[quarter].ap().opt()],
                    outs=[xqg[quarter].ap().opt()])

            wblk = qTp.tile([128, 8192], F16, tag="wblk", name="wblk")
            wblk4 = wblk[:].rearrange("p (t s q) -> p t s q", t=16, s=4)

            def w_chunk(ch, rbw):
                u, hv = ch // 2, ch % 2
                qc = quant_chunk(
                    wrB, rbw,
                    lambda c: mk[(c // 2) * 128:(c // 2 + 1) * 128,
                                 (c % 2) * 1024:(c % 2 + 1) * 1024], ch,
                    qtmp, psp, "pst")
                for q2 in range(2):
                    pst = psp.tile([128, 512], DT, tag="pst", name="pstw")
                    for r in range(4):
                        p = q2 * 4 + r
                        nc.tensor.matmul(
                            pst[:, r * 128:(r + 1) * 128],
                            qc[:, p * 128:(p + 1) * 128], h128h[:],
                            start=True, stop=True)
                    v0 = hv * 8 + q2 * 4
                    dst = wblk4[:, v0:v0 + 4, u, :]
                    src = pst[:].rearrange("p (r q) -> p r q", r=4)
                    if ch % 2 == 0:
                        nc.vector.tensor_copy(dst, src)
                    else:
                        nc.scalar.copy(dst, src)

            # x quarters gate the AllGather train (critical path): emit all
            # four first so the AG triggers only depend on the x quant
            # chain.  w chunks follow (i-half 0 first, so early GEMM
            # k-tiles are ready sooner); the GEMM can't start before wblk
            # regardless.
            x_quarter(0, qtmp, psp, "pst", 4)
            sgw, rbw = scale_finish("w", sw_o)
            w_chunk(0, rbw)
            x_quarter(1, qtmp, psp, "pst", 4)
            w_chunk(2, rbw)
            w_chunk(4, rbw)
            x_quarter(2, qtmp, psp, "pst", 4)
            w_chunk(6, rbw)
            w_chunk(1, rbw)
            x_quarter(3, qtmp, psp, "pst", 4)
            w_chunk(3, rbw)
            w_chunk(5, rbw)
            w_chunk(7, rbw)

            # alpha = sx*sw/(QMAX^2 * 2^24) = 1/(rbx*rbw*2^24)
            alp = qsc.tile([1, 1], DT, tag="alp", name="alp")
            nc.vector.tensor_tensor(alp[0:1, 0:1], sgx[0:1, 0:1],
                                    sgw[0:1, 0:1], op=A.mult)
            nc.vector.tensor_scalar_mul(alp[0:1, 0:1], alp[0:1, 0:1],
                                        float(1 << 24))
            al = qsc.tile([1, 1], DT, tag="al", name="al")
            nc.vector.reciprocal(al[0:1, 0:1], alp[0:1, 0:1])
            alb = qsc.tile([128, 1], DT, tag="alb", name="alb")
            nc.gpsimd.partition_broadcast(alb[:, 0:1], al[0:1, 0:1])

        # ---------- GEMM (output is final up to alpha) ----------------
        with tc.tile_pool(name="gem", bufs=8) as gem, \
             tc.tile_pool(name="gps", bufs=8, space="PSUM") as gps:
            wblk_g = wblk  # keep referenced
            for g in range(4):
                psY = [gps.tile([128, 512], DT, tag="gp",
                                name=f"gpt{g}_{j}", bufs=8)
                       for j in range(8)]
                for t in range(16):
                    xt = gem.tile([128, 1024], F16, tag="xt", name="xtt")
                    nc.sync.dma_start(
                        xt[:], xqg[g][(t // 2) * 256 + (t % 2) * 128:
                                      (t // 2) * 256 + (t % 2) * 128 + 128,
                                      :])
                    for s in range(4):
                        for hb in range(2):
                            nc.tensor.matmul(
                                psY[s * 2 + hb][:],
                                wblk_g[:, t * 512 + s * 128:
                                       t * 512 + (s + 1) * 128],
                                xt[:, hb * 512:(hb + 1) * 512],
                                start=(t == 0), stop=(t == 15))
                for s in range(4):
                    for hb in range(2):
                        ot = gem.tile([128, 512], F16, tag="ot",
                                      name="ott", bufs=4)
                        nc.vector.tensor_scalar(
                            ot[:], psY[s * 2 + hb][:], alb[:, 0:1], None,
                            op0=A.mult)
                        nc.sync.dma_start(
                            out[s * 128:(s + 1) * 128,
                                g * 1024 + hb * 512:
                                g * 1024 + (hb + 1) * 512], ot[:])
    nc.compile()
    return nc


def make_in_maps(inputs):
    H32 = _sylvester(32)
    x = np.asarray(inputs["inputs"], np.float32)
    w = np.asarray(inputs["kernel"], np.float32)
    nxp = (0.5 - np.asarray(inputs["noise_x"], np.float32))
    nwp = (0.5 - np.asarray(inputs["noise_w"], np.float32))

    # host cross-shard combines (H32 factors)
    xhf = np.einsum('st,bti->bsi', H32, x.reshape(128, 32, IN))
    whf = np.einsum('st,itp->isp', H32, w.reshape(IN, 32, 128))
    nx3 = nxp.reshape(128, 32, IN)

    in_maps = []
    for k in range(NCORES):
        xs = np.ascontiguousarray(xhf[:, :, k * CS:(k + 1) * CS]) \
               .reshape(128, 8192).astype(FP16)
        nks = np.ascontiguousarray(nx3[:, :, k * CS:(k + 1) * CS]) \
                .reshape(128, 8192).astype(FP16)
        ws = np.ascontiguousarray(whf[:, 4 * k:4 * k + 4, :]
                                  .transpose(1, 2, 0)) \
               .reshape(512, IN).astype(FP16)
        mks = np.ascontiguousarray(
            nwp[:, k * FS:(k + 1) * FS].T).astype(FP16)
        in_maps.append({"xh": xs, "nk": nks, "wh": ws, "mk": mks})
    return in_maps


def kernel(**inputs):
    from concourse.bass_utils import run_bass_kernel_spmd

    if "nc" not in _cache:
        _cache["nc"] = _build()
    nc = _cache["nc"]

    bias = np.asarray(inputs["bias"], np.float32)
    in_maps = make_in_maps(inputs)

    res = run_bass_kernel_spmd(nc, in_maps, list(range(NCORES)))

    # host unshard: H32 mirror factors over feature-blocks and batch-low
    H32 = _sylvester(32)
    V = np.stack([r["out"].astype(np.float32) for r in res.results])
    V = V.reshape(NCORES, 4, 128, 32, 128)        # [a, u, q, b2, b1]
    V = V.reshape(32, 128, 32, 128)               # [g=(a,u), q, b2, b1]
    V = np.einsum('st,tqbj->sqbj', H32, V)        # H32 over feature blocks
    V = np.einsum('cd,sqdj->sqcj', H32, V)        # H32 over batch-low
    y = V.transpose(3, 2, 0, 1).reshape(B, F)     # [b1, b2, g, q] -> [B, F]
    return (y + bias[None, :]).astype(np.float32)
